# revision 1
# baseline (speedup 1.0000x reference)
"""Conv2D 3x3 (stride 1, pad 1) NCHW/OIHW, data-parallel over 8 NeuronCores.

Full inputs: x (16,32,224,224) f32, weight (64,32,3,3) f32, bias (64,) f32.
Full output: (16,64,224,224) f32.

Raw-Bass SPMD kernel, per core (2 images), per 28-row block:
  - One 128-partition staged input DMA: XS[p = rh*64 + img*32 + ic, s, c] =
    xpad[img, ic, i0 + rh*15 + s, c] (full SBUF port width).
  - DVE redistributes XS into per-image im2col buffers XB[96, 30, 226]
    (group g at slot s holds padded row i0+s+g): 2 copies for group 0
    (the two rh halves), then 2 shifted copies for groups 1/2.
  - Each output row-pair = 3 PSUM-accumulated matmuls (K=96, M=64, N=448),
    dx realized as a free-dim offset.  The two images ride different PE
    column groups (PSUM partitions 0-63 / 64-127) and overlap in the array.
  - ScalarE evacuates PSUM + bias -> OUT[128, 28, 224] (both images at
    once); SP issues two 128-partition output DMAs per block.
  - All cross-engine sync is explicit semaphores; every DMA semaphore has
    at most one DMA in flight and consumers wait for its full count (sound
    under out-of-order per-SDMA-engine completion).  The local walrus
    rejects multi-wait instructions, which rules out TileContext codegen.
"""

import sys

sys.path.insert(0, "/opt/trn_rl_repo")

from contextlib import ExitStack

import numpy as np

import concourse.bass as bass
from concourse import mybir
from concourse.bass_utils import run_bass_kernel_spmd

N_CORES = 8
IMGS_PER_CORE = 2
IC, OC, H, W = 32, 64, 224, 224
HP, WP = 226, 226  # padded
BLK = 28  # output rows per block
N_BLK = H // BLK
PPB = BLK // 2  # row-pairs per block (14)
RH = (BLK + 2) // 2  # rows per rh-half in the staged load (15)
XR = 3  # staging + xb ring depth
OR = 2  # out ring depth
NPS = 8  # psum banks in rotation

# "f32": exact fp32 matmul (slow but bit-safe).  "bf16": inputs cast to bf16
# on host (half input DMA, full-rate PE, 4x DVE copies).
DT_MODE = "bf16"

TRACE = False  # test.py can flip this to get LAST_EXEC_NS
LAST_EXEC_NS = None
LAST_RESULTS = None

_nc_cache = {}


def _install_ntff_shim():
    """The agent image's antenv lacks axon_hooks; recreate the NTFF profile
    hook via ctypes against libaxon_pjrt.so (same ABI trn_boot.py uses)."""
    try:
        import antenv.axon_hooks  # noqa: F401

        return
    except ImportError:
        pass
    import contextlib
    import ctypes
    import types

    so_path = "/opt/axon/libaxon_pjrt.so"
    lib = ctypes.CDLL(so_path)
    if not hasattr(lib, "axon_start_nrt_profile"):
        return
    lib.axon_start_nrt_profile.argtypes = [
        ctypes.POINTER(ctypes.c_int64),
        ctypes.c_size_t,
    ]
    lib.axon_start_nrt_profile.restype = ctypes.c_int64
    lib.axon_stop_nrt_profile.argtypes = [ctypes.c_char_p]
    lib.axon_stop_nrt_profile.restype = ctypes.c_int64

    @contextlib.contextmanager
    def _hook(output_dir, device_ids):
        import jax

        jax.devices()
        if device_ids:
            ids = (ctypes.c_int64 * len(device_ids))(*device_ids)
            rc = lib.axon_start_nrt_profile(ids, len(device_ids))
        else:
            rc = lib.axon_start_nrt_profile(None, 0)
        if rc != 0:
            raise RuntimeError(f"axon_start_nrt_profile rc={rc}")
        try:
            yield
        finally:
            n = lib.axon_stop_nrt_profile(str(output_dir).encode())
            print(f"ntff profile: {n} file(s) written to {output_dir}")

    mod = types.ModuleType("antenv.axon_hooks")
    mod.get_axon_ntff_profile_hook = lambda: _hook
    mod.set_axon_ntff_profile_hook = lambda h: None
    import antenv

    sys.modules["antenv.axon_hooks"] = mod
    antenv.axon_hooks = mod


def _build_nc(mode: str) -> bass.Bass:
    f32 = mybir.dt.float32
    in_dt = mybir.dt.bfloat16 if mode == "bf16" else f32

    nc = bass.Bass()
    x = nc.dram_tensor("x", [IMGS_PER_CORE, IC, HP, WP], in_dt, kind="ExternalInput")
    wt = nc.dram_tensor("wt", [96, 3, OC], in_dt, kind="ExternalInput")
    bias = nc.dram_tensor("bias", [128, 1], f32, kind="ExternalInput")
    y = nc.dram_tensor("y", [IMGS_PER_CORE, OC, H, W], f32, kind="ExternalOutput")

    ctx = ExitStack()
    wt_sb = ctx.enter_context(nc.sbuf_tensor("wt_sb", [96, 3, OC], in_dt))
    bias_sb = ctx.enter_context(nc.sbuf_tensor("bias_sb", [128, 1], f32))
    xs = [
        ctx.enter_context(nc.sbuf_tensor(f"xs_{r}", [128, RH, WP], in_dt))
        for r in range(XR)
    ]
    xb = [
        [
            ctx.enter_context(nc.sbuf_tensor(f"xb_{i}_{r}", [96, BLK + 2, WP], in_dt))
            for r in range(XR)
        ]
        for i in range(IMGS_PER_CORE)
    ]
    outb = [
        ctx.enter_context(nc.sbuf_tensor(f"outb_{s}", [128, BLK, W], f32))
        for s in range(OR)
    ]
    ps = [
        ctx.enter_context(nc.psum_tensor(f"ps_{k}", [128, 2, W], f32))
        for k in range(NPS)
    ]

    s_wt = ctx.enter_context(nc.semaphore("s_wt"))
    s_bias = ctx.enter_context(nc.semaphore("s_bias"))
    s_xs = [ctx.enter_context(nc.semaphore(f"s_xs_{r}")) for r in range(XR)]
    s_yo = [
        [ctx.enter_context(nc.semaphore(f"s_yo_{s}_{h}")) for h in range(2)]
        for s in range(OR)
    ]
    s_cp = ctx.enter_context(nc.semaphore("s_cp"))
    s_mm = ctx.enter_context(nc.semaphore("s_mm"))
    s_ev = ctx.enter_context(nc.semaphore("s_ev"))

    st_img = IC * HP * WP
    st_ic = HP * WP

    def staged_src(i0):
        # partition p = rh*64 + img*32 + ic ; free (s, c)
        return bass.AP(
            tensor=x[0, 0, 0:1, 0:1].tensor,
            offset=i0 * WP,
            ap=[[RH * WP, 2], [st_img, 2], [st_ic, IC], [WP, RH], [1, WP]],
        )

    with ctx, nc.Block() as block:

        @block.sync
        def _(sync):
            def emit_out(b):
                i0 = b * BLK
                sync.wait_ge(s_ev, PPB * (b + 1))
                ob = outb[b % OR]
                for h in range(2):
                    sync.dma_start(
                        out=y[:, :, i0 + h * PPB : i0 + (h + 1) * PPB, :],
                        in_=ob[:, h * PPB : (h + 1) * PPB, :],
                    ).then_inc(s_yo[b % OR][h], 16)

            sync.dma_start(out=wt_sb[:, :, :], in_=wt[:, :, :]).then_inc(s_wt, 16)
            sync.dma_start(out=bias_sb[:, :], in_=bias[:, :]).then_inc(s_bias, 16)
            for b in range(N_BLK):
                # input load for block b (XS slot b%XR)
                if b >= XR:
                    # XS slot reuse: redistribution copies of block b-XR done
                    sync.wait_ge(s_cp, 8 * (b - XR) + 4)
                sync.dma_start(out=xs[b % XR].ap(), in_=staged_src(b * BLK)).then_inc(
                    s_xs[b % XR], 16
                )
                # output stores for block b-1 (keeps SP one block ahead)
                if b >= 1:
                    emit_out(b - 1)
            emit_out(N_BLK - 1)
            for s in range(OR):
                n_uses = len([bb for bb in range(N_BLK) if bb % OR == s])
                for h in range(2):
                    sync.wait_ge(s_yo[s][h], 16 * n_uses)

        @block.vector
        def _(v):
            for b in range(N_BLK):
                r = b % XR
                v.wait_ge(s_xs[r], 16 * (b // XR + 1))
                if b >= XR:
                    # xb slot reuse: PE matmuls of block b-XR done
                    v.wait_ge(s_mm, PPB * (b - XR + 1))
                for img in range(IMGS_PER_CORE):
                    t = xb[img][r]
                    # group 0 from the two rh-halves of the staging buffer
                    for rh in range(2):
                        v.tensor_copy(
                            out=t[0:32, rh * RH : (rh + 1) * RH, :],
                            in_=xs[r][rh * 64 + img * 32 : rh * 64 + img * 32 + 32],
                        ).then_inc(s_cp, 1)
                # group-0 writes must be visible before the shifted reads
                # (same-engine, but the DVE write pipeline is deep)
                v.wait_ge(s_cp, 8 * b + 4)
                for img in range(IMGS_PER_CORE):
                    t = xb[img][r]
                    # groups 1/2 = group 0 shifted down one/two rows
                    v.tensor_copy(
                        out=t[32:64, 0:BLK, :], in_=t[0:32, 1 : BLK + 1, :]
                    ).then_inc(s_cp, 1)
                    v.tensor_copy(
                        out=t[64:96, 0:BLK, :], in_=t[0:32, 2 : BLK + 2, :]
                    ).then_inc(s_cp, 1)

        @block.tensor
        def _(t):
            t.wait_ge(s_wt, 16)
            for b in range(N_BLK):
                t.wait_ge(s_cp, 8 * (b + 1))
                for p in range(PPB):
                    gp = b * PPB + p
                    if gp >= NPS:
                        t.wait_ge(s_ev, gp - NPS + 1)
                    bank = ps[gp % NPS]
                    b0 = 2 * p
                    last = None
                    for dx in range(3):
                        for img in range(IMGS_PER_CORE):
                            last = nc.tensor.matmul(
                                bank[img * OC : (img + 1) * OC, :, :],
                                wt_sb[:, dx, :],
                                xb[img][b % XR][:, b0 : b0 + 2, dx : dx + W],
                                start=dx == 0,
                                stop=dx == 2,
                                skip_group_check=True,
                            )
                    last.then_inc(s_mm, 1)

        @block.scalar
        def _(sc):
            sc.wait_ge(s_bias, 16)
            for b in range(N_BLK):
                if b >= OR:
                    for h in range(2):
                        sc.wait_ge(s_yo[b % OR][h], 16 * ((b - OR) // OR + 1))
                ob = outb[b % OR]
                for p in range(PPB):
                    gp = b * PPB + p
                    sc.wait_ge(s_mm, gp + 1)
                    sc.activation(
                        ob[:, 2 * p : 2 * p + 2, :],
                        ps[gp % NPS][:, :, :],
                        mybir.ActivationFunctionType.Identity,
                        bias=bias_sb[:, :],
                    ).then_inc(s_ev, 1)

    return nc


def _get_nc(mode: str) -> bass.Bass:
    if mode not in _nc_cache:
        _nc_cache[mode] = _build_nc(mode)
    return _nc_cache[mode]


def kernel(x: np.ndarray, weight: np.ndarray, bias: np.ndarray) -> np.ndarray:
    global LAST_EXEC_NS, LAST_RESULTS
    mode = DT_MODE
    n = x.shape[0]
    assert n == N_CORES * IMGS_PER_CORE

    if mode == "bf16":
        import ml_dtypes

        in_np = ml_dtypes.bfloat16
    else:
        in_np = np.float32

    xp = np.zeros((n, IC, HP, WP), dtype=in_np)
    xp[:, :, 1 : H + 1, 1 : W + 1] = x
    # WT[dy*32+ic, dx, oc] = weight[oc, ic, dy, dx]
    wt = np.ascontiguousarray(weight.transpose(2, 1, 3, 0).reshape(96, 3, OC)).astype(
        in_np
    )
    b2 = np.ascontiguousarray(np.tile(bias.reshape(OC, 1), (2, 1))).astype(np.float32)

    nc = _get_nc(mode)
    in_maps = [
        {
            "x": np.ascontiguousarray(xp[i * IMGS_PER_CORE : (i + 1) * IMGS_PER_CORE]),
            "wt": wt,
            "bias": b2,
        }
        for i in range(N_CORES)
    ]
    if TRACE:
        _install_ntff_shim()
    res = run_bass_kernel_spmd(nc, in_maps, core_ids=list(range(N_CORES)), trace=TRACE)
    LAST_EXEC_NS = res.exec_time_ns
    LAST_RESULTS = res
    y = np.concatenate([r["y"] for r in res.results], axis=0)
    return y.astype(np.float32)



# revision 5
# speedup vs baseline: 1.3016x; 1.3016x over previous
"""Conv2D 3x3 (stride 1, pad 1) NCHW/OIHW, data-parallel over 8 NeuronCores.

Full inputs: x (16,32,224,224) f32, weight (64,32,3,3) f32, bias (64,) f32.
Full output: (16,64,224,224) f32.

Raw-Bass SPMD kernel, per core (2 images), per 28-row block:
  - The im2col buffer XB[img][p = dy*32 + ic, s, c] = xpad[img, ic, i0+s+dy, c]
    is DMA'd DIRECTLY from HBM, one dma_start per (img, dy).  Each load's
    DRAM-side AP has the 32-wide ic dim outermost, so the descriptor
    generator sprays it round-robin across all 16 SDMA engines (the
    outermost source dim is what gets split); each partition reads a
    single ~12.6KB contiguous row range.  With USE_DVE_DY2, the dy=2
    replica is instead one wide DVE copy from the dy=1 group (which loads
    29 row-slots so slot s+1 covers row i0+2+s), cutting HBM input
    traffic from 3x to 2x of the unique bytes.
  - Each output row-pair = 3 PSUM-accumulated matmuls (K=96, M=64, N=448),
    dx realized as a free-dim offset.  The two images ride different PE
    column groups (PSUM partitions 0-63 / 64-127) and overlap in the array.
  - PSUM evacuation (+bias, f32->bf16) is split across two engines:
    ScalarE takes even row-pairs (activation w/ bias), DVE takes odd ones
    (tensor_scalar_add w/ per-partition bias).  Output rides HBM as bf16
    (halves store traffic; measured end-to-end rel err ~3.6e-3 vs the 2e-2
    gate) and is widened to f32 on the host after the gather.
  - All cross-engine sync is explicit semaphores; each DMA semaphore's
    waiters always wait for the full +16-per-DMA count (sound under
    out-of-order per-SDMA-engine completion).  The local walrus rejects
    multi-wait instructions, so every wait is its own instruction.
"""

import sys

sys.path.insert(0, "/opt/trn_rl_repo")

from contextlib import ExitStack

import numpy as np

import concourse.bass as bass
from concourse import mybir
from concourse.bass_utils import run_bass_kernel_spmd

N_CORES = 8
IMGS_PER_CORE = 2
IC, OC, H, W = 32, 64, 224, 224
HP, WP = 226, 226  # padded
BLK = 28  # output rows per block
N_BLK = H // BLK
PPB = BLK // 2  # row-pairs per block (14)
SLOTS = BLK + 1  # xb row-slots (dy=1 group loads one extra row for the dy=2 copy)
XR = 3  # xb ring depth
OR = 2  # out ring depth
NPS = 8  # psum banks in rotation

# dy=2 im2col replica via one wide DVE copy instead of a third HBM read.
USE_DVE_DY2 = True

DT_MODE = "bf16"  # kept for test.py compat; only bf16 is supported

TRACE = False  # test.py can flip this to get LAST_EXEC_NS
LAST_EXEC_NS = None
LAST_RESULTS = None

_nc_cache = {}


def _install_ntff_shim():
    """The agent image's antenv lacks axon_hooks; recreate the NTFF profile
    hook via ctypes against libaxon_pjrt.so (same ABI trn_boot.py uses)."""
    try:
        import antenv.axon_hooks  # noqa: F401

        return
    except ImportError:
        pass
    import contextlib
    import ctypes
    import types

    so_path = "/opt/axon/libaxon_pjrt.so"
    lib = ctypes.CDLL(so_path)
    if not hasattr(lib, "axon_start_nrt_profile"):
        return
    lib.axon_start_nrt_profile.argtypes = [
        ctypes.POINTER(ctypes.c_int64),
        ctypes.c_size_t,
    ]
    lib.axon_start_nrt_profile.restype = ctypes.c_int64
    lib.axon_stop_nrt_profile.argtypes = [ctypes.c_char_p]
    lib.axon_stop_nrt_profile.restype = ctypes.c_int64

    @contextlib.contextmanager
    def _hook(output_dir, device_ids):
        import jax

        jax.devices()
        if device_ids:
            ids = (ctypes.c_int64 * len(device_ids))(*device_ids)
            rc = lib.axon_start_nrt_profile(ids, len(device_ids))
        else:
            rc = lib.axon_start_nrt_profile(None, 0)
        if rc != 0:
            raise RuntimeError(f"axon_start_nrt_profile rc={rc}")
        try:
            yield
        finally:
            n = lib.axon_stop_nrt_profile(str(output_dir).encode())
            print(f"ntff profile: {n} file(s) written to {output_dir}")

    mod = types.ModuleType("antenv.axon_hooks")
    mod.get_axon_ntff_profile_hook = lambda: _hook
    mod.set_axon_ntff_profile_hook = lambda h: None
    import antenv

    sys.modules["antenv.axon_hooks"] = mod
    antenv.axon_hooks = mod


# Evacuation split: ScalarE takes even row-pairs, DVE odd ones.
def _cnt_s(gp):
    """# of ScalarE evacs among global row-pairs 0..gp (gp even)."""
    b, p = divmod(gp, PPB)
    return 7 * b + p // 2 + 1


def _cnt_v(gp):
    """# of DVE evacs among global row-pairs 0..gp (gp odd)."""
    b, p = divmod(gp, PPB)
    return 7 * b + (p - 1) // 2 + 1


def _build_nc() -> bass.Bass:
    f32 = mybir.dt.float32
    bf16 = mybir.dt.bfloat16
    n_dy_dma = 2 if USE_DVE_DY2 else 3

    nc = bass.Bass()
    x = nc.dram_tensor("x", [IMGS_PER_CORE, IC, HP, WP], bf16, kind="ExternalInput")
    wt = nc.dram_tensor("wt", [96, 3, OC], bf16, kind="ExternalInput")
    bias = nc.dram_tensor("bias", [128, 1], f32, kind="ExternalInput")
    y = nc.dram_tensor("y", [IMGS_PER_CORE, OC, H, W], bf16, kind="ExternalOutput")

    ctx = ExitStack()
    wt_sb = ctx.enter_context(nc.sbuf_tensor("wt_sb", [96, 3, OC], bf16))
    bias_sb = ctx.enter_context(nc.sbuf_tensor("bias_sb", [128, 1], f32))
    xb = [
        [
            ctx.enter_context(nc.sbuf_tensor(f"xb_{i}_{r}", [96, SLOTS, WP], bf16))
            for r in range(XR)
        ]
        for i in range(IMGS_PER_CORE)
    ]
    outb = [
        ctx.enter_context(nc.sbuf_tensor(f"outb_{s}", [128, BLK, W], bf16))
        for s in range(OR)
    ]
    ps = [
        ctx.enter_context(nc.psum_tensor(f"ps_{k}", [128, 2, W], f32))
        for k in range(NPS)
    ]

    s_wt = ctx.enter_context(nc.semaphore("s_wt"))
    s_bias = ctx.enter_context(nc.semaphore("s_bias"))
    s_xb = [
        [ctx.enter_context(nc.semaphore(f"s_xb_{i}_{r}")) for r in range(XR)]
        for i in range(IMGS_PER_CORE)
    ]
    s_yo = [
        [ctx.enter_context(nc.semaphore(f"s_yo_{s}_{h}")) for h in range(2)]
        for s in range(OR)
    ]
    s_cp = ctx.enter_context(nc.semaphore("s_cp"))
    s_mm = ctx.enter_context(nc.semaphore("s_mm"))
    s_evs = ctx.enter_context(nc.semaphore("s_evs"))
    s_evv = ctx.enter_context(nc.semaphore("s_evv"))

    st_img = IC * HP * WP
    st_ic = HP * WP

    def dy_src(img, i0, dy, n_rows):
        # partition = ic (32-wide, outermost -> 16-engine DMA spray);
        # free (s, c); each partition reads n_rows*WP contiguous elements.
        return bass.AP(
            tensor=x[0, 0, 0:1, 0:1].tensor,
            offset=img * st_img + (i0 + dy) * WP,
            ap=[[st_ic, IC], [WP, n_rows], [1, WP]],
        )

    with ctx, nc.Block() as block:

        @block.sync
        def _(sync):
            def emit_out(b):
                i0 = b * BLK
                ob = outb[b % OR]
                # h=0: row-pairs 0..6 (scalar evacs 4, dve 3); h=1: 7..13.
                sync.wait_ge(s_evs, 7 * b + 4)
                sync.wait_ge(s_evv, 7 * b + 3)
                sync.dma_start(
                    out=y[:, :, i0 : i0 + PPB, :],
                    in_=ob[:, 0:PPB, :],
                ).then_inc(s_yo[b % OR][0], 16)
                sync.wait_ge(s_evs, 7 * (b + 1))
                sync.wait_ge(s_evv, 7 * (b + 1))
                sync.dma_start(
                    out=y[:, :, i0 + PPB : i0 + BLK, :],
                    in_=ob[:, PPB:BLK, :],
                ).then_inc(s_yo[b % OR][1], 16)

            sync.dma_start(out=wt_sb[:, :, :], in_=wt[:, :, :]).then_inc(s_wt, 16)
            sync.dma_start(out=bias_sb[:, :], in_=bias[:, :]).then_inc(s_bias, 16)
            for b in range(N_BLK):
                r = b % XR
                i0 = b * BLK
                if b >= XR:
                    # xb slot reuse: PE matmuls of block b-XR done
                    sync.wait_ge(s_mm, PPB * (b - XR + 1))
                    if USE_DVE_DY2:
                        # and the dy=2 copies of block b-XR done (DMA would
                        # overwrite their dy=1 source rows)
                        sync.wait_ge(s_cp, IMGS_PER_CORE * (b - XR + 1))
                for img in range(IMGS_PER_CORE):
                    for dy in range(n_dy_dma):
                        n_rows = SLOTS if dy == 1 else BLK
                        sync.dma_start(
                            out=xb[img][r][dy * 32 : (dy + 1) * 32, 0:n_rows, :],
                            in_=dy_src(img, i0, dy, n_rows),
                        ).then_inc(s_xb[img][r], 16)
                # output stores for block b-1 (keeps SP one block ahead)
                if b >= 1:
                    emit_out(b - 1)
            emit_out(N_BLK - 1)
            for s in range(OR):
                n_uses = len([bb for bb in range(N_BLK) if bb % OR == s])
                for h in range(2):
                    sync.wait_ge(s_yo[s][h], 16 * n_uses)

        def _evac_block(eng, b, is_scalar):
            ob = outb[b % OR]
            my_ps = [p for p in range(PPB) if (p % 2 == 0) == is_scalar]
            for p in my_ps:
                gp = b * PPB + p
                # outb slot reuse: stores of block b-OR done, per half
                if b >= OR:
                    if p == my_ps[0]:
                        eng.wait_ge(s_yo[b % OR][0], 16 * ((b - OR) // OR + 1))
                    if p == (8 if is_scalar else 7):
                        eng.wait_ge(s_yo[b % OR][1], 16 * ((b - OR) // OR + 1))
                eng.wait_ge(s_mm, gp + 1)
                if is_scalar:
                    eng.activation(
                        ob[:, 2 * p : 2 * p + 2, :],
                        ps[gp % NPS][:, :, :],
                        mybir.ActivationFunctionType.Identity,
                        bias=bias_sb[:, :],
                    ).then_inc(s_evs, 1)
                else:
                    eng.tensor_scalar_add(
                        ob[:, 2 * p : 2 * p + 2, :],
                        ps[gp % NPS][:, :, :],
                        bias_sb[:, :],
                    ).then_inc(s_evv, 1)

        if USE_DVE_DY2:

            @block.vector
            def _(v):
                # dy=2 im2col replicas: xb[64:96, s] = xb[32:64, s+1]
                # (dy=1 loads SLOTS=29 rows so the shift stays in-buffer).
                # DVE also evacuates odd row-pairs; evacs of block b-1 are
                # interleaved AFTER copies of block b so PE never stalls on
                # the DVE evac tail when crossing a block boundary.
                for b in range(N_BLK):
                    r = b % XR
                    for img in range(IMGS_PER_CORE):
                        v.wait_ge(s_xb[img][r], 16 * n_dy_dma * (b // XR + 1))
                    if b >= XR:
                        # write-after-read: PE of block b-XR reads old dy=2
                        v.wait_ge(s_mm, PPB * (b - XR + 1))
                    for img in range(IMGS_PER_CORE):
                        v.tensor_copy(
                            out=xb[img][r][64:96, 0:BLK, :],
                            in_=xb[img][r][32:64, 1 : BLK + 1, :],
                        ).then_inc(s_cp, 1)
                    if b >= 1:
                        _evac_block(v, b - 1, is_scalar=False)
                _evac_block(v, N_BLK - 1, is_scalar=False)

        else:

            @block.vector
            def _(v):
                for b in range(N_BLK):
                    _evac_block(v, b, is_scalar=False)

        @block.tensor
        def _(t):
            t.wait_ge(s_wt, 16)
            for b in range(N_BLK):
                r = b % XR
                for img in range(IMGS_PER_CORE):
                    t.wait_ge(s_xb[img][r], 16 * n_dy_dma * (b // XR + 1))
                if USE_DVE_DY2:
                    t.wait_ge(s_cp, IMGS_PER_CORE * (b + 1))
                for p in range(PPB):
                    gp = b * PPB + p
                    if gp >= NPS:
                        # psum bank reuse: evac of row-pair gp-NPS done
                        # (NPS=8 even, PPB=14 even -> same parity as gp)
                        tgt = gp - NPS
                        if tgt % 2 == 0:
                            t.wait_ge(s_evs, _cnt_s(tgt))
                        else:
                            t.wait_ge(s_evv, _cnt_v(tgt))
                    bank = ps[gp % NPS]
                    b0 = 2 * p
                    last = None
                    for dx in range(3):
                        for img in range(IMGS_PER_CORE):
                            last = nc.tensor.matmul(
                                bank[img * OC : (img + 1) * OC, :, :],
                                wt_sb[:, dx, :],
                                xb[img][r][:, b0 : b0 + 2, dx : dx + W],
                                start=dx == 0,
                                stop=dx == 2,
                                skip_group_check=True,
                            )
                    last.then_inc(s_mm, 1)

        @block.scalar
        def _(sc):
            sc.wait_ge(s_bias, 16)
            for b in range(N_BLK):
                _evac_block(sc, b, is_scalar=True)

    return nc


def _get_nc() -> bass.Bass:
    if "nc" not in _nc_cache:
        _nc_cache["nc"] = _build_nc()
    return _nc_cache["nc"]


def kernel(x: np.ndarray, weight: np.ndarray, bias: np.ndarray) -> np.ndarray:
    global LAST_EXEC_NS, LAST_RESULTS
    import ml_dtypes

    n = x.shape[0]
    assert n == N_CORES * IMGS_PER_CORE

    in_np = ml_dtypes.bfloat16
    xp = np.zeros((n, IC, HP, WP), dtype=in_np)
    xp[:, :, 1 : H + 1, 1 : W + 1] = x
    # WT[dy*32+ic, dx, oc] = weight[oc, ic, dy, dx]
    wt = np.ascontiguousarray(weight.transpose(2, 1, 3, 0).reshape(96, 3, OC)).astype(
        in_np
    )
    b2 = np.ascontiguousarray(np.tile(bias.reshape(OC, 1), (2, 1))).astype(np.float32)

    nc = _get_nc()
    in_maps = [
        {
            "x": np.ascontiguousarray(xp[i * IMGS_PER_CORE : (i + 1) * IMGS_PER_CORE]),
            "wt": wt,
            "bias": b2,
        }
        for i in range(N_CORES)
    ]
    if TRACE:
        _install_ntff_shim()
    res = run_bass_kernel_spmd(nc, in_maps, core_ids=list(range(N_CORES)), trace=TRACE)
    LAST_EXEC_NS = res.exec_time_ns
    LAST_RESULTS = res
    y = np.concatenate([r["y"] for r in res.results], axis=0)
    return y.astype(np.float32)


# revision 6
# speedup vs baseline: 2.1949x; 1.6864x over previous
"""Conv2D 3x3 (stride 1, pad 1) NCHW/OIHW, data-parallel over 8 NeuronCores.

Full inputs: x (16,32,224,224) f32, weight (64,32,3,3) f32, bias (64,) f32.
Full output: (16,64,224,224) f32.

Raw-Bass SPMD kernel, per core (2 images), per 28-row block:
  - The im2col buffer XB[img][p = dy*32 + ic, s, c] = xpad[img, ic, i0+s+dy, c]
    for dy=0,1 is DMA'd DIRECTLY from HBM.  Each load's DRAM-side AP has the
    32-wide ic dim outermost, so the descriptor generator sprays it
    round-robin across all 16 SDMA engines (the outermost source dim is what
    gets split).  Loads are split into <=15-row pieces so every DMA packet
    stays <=6.8KB: the SDMA engines move ~25GB/s on ~6KB packets but drop to
    ~14GB/s on >8KB packets.  The dy=2 replica is one wide DVE copy from the
    dy=1 group (which loads 29 row-slots so slot s+1 covers row i0+2+s),
    keeping HBM input traffic at 2x the unique bytes instead of 3x.
  - Engine assignment decouples the three pipelines so PE never waits on a
    chain that leads back through input-DMA arrival:
      GpSimd: issues all input DMAs (its own hardware queue, separate from
              the store queue, so loads never sit behind stores in a FIFO).
      DVE:    dy=2 copies for block b, then PSUM evac of row-pairs 7..13 of
              block b-1 (tensor_scalar_add w/ per-partition bias, f32->bf16).
      Scalar: PSUM evac of row-pairs 0..6 (activation w/ bias).
      SP:     output stores; each store half waits on exactly one evac sem.
  - Each output row-pair = 3 PSUM-accumulated matmuls (K=96, M=64, N=448),
    dx realized as a free-dim offset.  The two images ride different PE
    column groups (PSUM partitions 0-63 / 64-127) and overlap in the array.
    This is SBUF->PE rhs-port bound (~28.9M rhs element reads / 128 lanes
    @2.8GHz ~= 80us per core), the structural floor for M=64.
  - Output rides HBM as bf16 (halves store traffic; measured end-to-end rel
    err ~3.6e-3 vs the 2e-2 gate) and is widened to f32 on the host.
  - All cross-engine sync is explicit semaphores; DMA-sem waiters always
    wait the full +16-per-DMA count (sound under out-of-order per-SDMA
    completion).  The local walrus rejects multi-wait instructions, so every
    wait is its own instruction.
"""

import sys

sys.path.insert(0, "/opt/trn_rl_repo")

from contextlib import ExitStack

import numpy as np

import concourse.bass as bass
from concourse import mybir
from concourse.bass_utils import run_bass_kernel_spmd

N_CORES = 8
IMGS_PER_CORE = 2
IC, OC, H, W = 32, 64, 224, 224
HP, WP = 226, 226  # padded
BLK = 28  # output rows per block
N_BLK = H // BLK
PPB = BLK // 2  # row-pairs per block (14)
SLOTS = BLK + 1  # xb row-slots (dy=1 group loads one extra row for the dy=2 copy)
XR = 3  # xb ring depth
OR = 2  # out ring depth
NPS = 8  # psum banks in rotation
EVS_PPB = 7  # row-pairs 0..6 per block evacuated by ScalarE
EVV_PPB = 7  # row-pairs 7..13 per block evacuated by DVE

DT_MODE = "bf16"  # kept for test.py compat; only bf16 is supported

TRACE = False  # test.py can flip this to get LAST_EXEC_NS
LAST_EXEC_NS = None
LAST_RESULTS = None

_nc_cache = {}


def _install_ntff_shim():
    """The agent image's antenv lacks axon_hooks; recreate the NTFF profile
    hook via ctypes against libaxon_pjrt.so (same ABI trn_boot.py uses)."""
    try:
        import antenv.axon_hooks  # noqa: F401

        return
    except ImportError:
        pass
    import contextlib
    import ctypes
    import types

    so_path = "/opt/axon/libaxon_pjrt.so"
    lib = ctypes.CDLL(so_path)
    if not hasattr(lib, "axon_start_nrt_profile"):
        return
    lib.axon_start_nrt_profile.argtypes = [
        ctypes.POINTER(ctypes.c_int64),
        ctypes.c_size_t,
    ]
    lib.axon_start_nrt_profile.restype = ctypes.c_int64
    lib.axon_stop_nrt_profile.argtypes = [ctypes.c_char_p]
    lib.axon_stop_nrt_profile.restype = ctypes.c_int64

    @contextlib.contextmanager
    def _hook(output_dir, device_ids):
        import jax

        jax.devices()
        if device_ids:
            ids = (ctypes.c_int64 * len(device_ids))(*device_ids)
            rc = lib.axon_start_nrt_profile(ids, len(device_ids))
        else:
            rc = lib.axon_start_nrt_profile(None, 0)
        if rc != 0:
            raise RuntimeError(f"axon_start_nrt_profile rc={rc}")
        try:
            yield
        finally:
            n = lib.axon_stop_nrt_profile(str(output_dir).encode())
            print(f"ntff profile: {n} file(s) written to {output_dir}")

    mod = types.ModuleType("antenv.axon_hooks")
    mod.get_axon_ntff_profile_hook = lambda: _hook
    mod.set_axon_ntff_profile_hook = lambda h: None
    import antenv

    sys.modules["antenv.axon_hooks"] = mod
    antenv.axon_hooks = mod


def _build_nc() -> bass.Bass:
    f32 = mybir.dt.float32
    bf16 = mybir.dt.bfloat16

    nc = bass.Bass()
    x = nc.dram_tensor("x", [IMGS_PER_CORE, IC, HP, WP], bf16, kind="ExternalInput")
    wt = nc.dram_tensor("wt", [96, 3, OC], bf16, kind="ExternalInput")
    bias = nc.dram_tensor("bias", [128, 1], f32, kind="ExternalInput")
    y = nc.dram_tensor("y", [IMGS_PER_CORE, OC, H, W], bf16, kind="ExternalOutput")

    ctx = ExitStack()
    wt_sb = ctx.enter_context(nc.sbuf_tensor("wt_sb", [96, 3, OC], bf16))
    bias_sb = ctx.enter_context(nc.sbuf_tensor("bias_sb", [128, 1], f32))
    xb = [
        [
            ctx.enter_context(nc.sbuf_tensor(f"xb_{i}_{r}", [96, SLOTS, WP], bf16))
            for r in range(XR)
        ]
        for i in range(IMGS_PER_CORE)
    ]
    outb = [
        ctx.enter_context(nc.sbuf_tensor(f"outb_{s}", [128, BLK, W], bf16))
        for s in range(OR)
    ]
    ps = [
        ctx.enter_context(nc.psum_tensor(f"ps_{k}", [128, 2, W], f32))
        for k in range(NPS)
    ]

    s_wt = ctx.enter_context(nc.semaphore("s_wt"))
    s_bias = ctx.enter_context(nc.semaphore("s_bias"))
    s_xb = [
        [ctx.enter_context(nc.semaphore(f"s_xb_{i}_{r}")) for r in range(XR)]
        for i in range(IMGS_PER_CORE)
    ]
    s_yo = [
        [ctx.enter_context(nc.semaphore(f"s_yo_{s}_{h}")) for h in range(2)]
        for s in range(OR)
    ]
    s_cp = ctx.enter_context(nc.semaphore("s_cp"))
    s_mm = ctx.enter_context(nc.semaphore("s_mm"))
    s_evs = ctx.enter_context(nc.semaphore("s_evs"))
    s_evv = ctx.enter_context(nc.semaphore("s_evv"))

    st_img = IC * HP * WP
    st_ic = HP * WP

    # (dy, slot0, n_rows) pieces: <=15 rows keeps every packet <=6.8KB.
    DMA_PIECES = [(0, 0, 14), (0, 14, 14), (1, 0, 15), (1, 15, 14)]

    def piece_src(img, i0, dy, s0, n_rows):
        # partition = ic (32-wide, outermost -> 16-engine DMA spray);
        # free (s, c); each partition reads n_rows*WP contiguous elements.
        return bass.AP(
            tensor=x[0, 0, 0:1, 0:1].tensor,
            offset=img * st_img + (i0 + dy + s0) * WP,
            ap=[[st_ic, IC], [WP, n_rows], [1, WP]],
        )

    with ctx, nc.Block() as block:

        @block.gpsimd
        def _(g):
            # input DMA issue: its own hardware queue, so input transfers
            # never queue behind output stores on the per-SDMA-engine FIFOs.
            for b in range(N_BLK):
                r = b % XR
                i0 = b * BLK
                if b >= XR:
                    # xb slot reuse: PE matmuls of block b-XR done (the dy=2
                    # copies of b-XR precede PE b-XR via s_cp, so this also
                    # covers the copy's read of the dy=1 rows).
                    g.wait_ge(s_mm, PPB * (b - XR + 1))
                for img in range(IMGS_PER_CORE):
                    for dy, s0, n_rows in DMA_PIECES:
                        g.dma_start(
                            out=xb[img][r][
                                dy * 32 : (dy + 1) * 32, s0 : s0 + n_rows, :
                            ],
                            in_=piece_src(img, i0, dy, s0, n_rows),
                        ).then_inc(s_xb[img][r], 16)

        @block.sync
        def _(sync):
            sync.dma_start(out=wt_sb[:, :, :], in_=wt[:, :, :]).then_inc(s_wt, 16)
            sync.dma_start(out=bias_sb[:, :], in_=bias[:, :]).then_inc(s_bias, 16)
            for b in range(N_BLK):
                i0 = b * BLK
                ob = outb[b % OR]
                # h=0: rows 0..13 = pairs 0..6, all ScalarE evacs
                sync.wait_ge(s_evs, EVS_PPB * (b + 1))
                sync.dma_start(
                    out=y[:, :, i0 : i0 + PPB, :],
                    in_=ob[:, 0:PPB, :],
                ).then_inc(s_yo[b % OR][0], 16)
                # h=1: rows 14..27 = pairs 7..13, all DVE evacs
                sync.wait_ge(s_evv, EVV_PPB * (b + 1))
                sync.dma_start(
                    out=y[:, :, i0 + PPB : i0 + BLK, :],
                    in_=ob[:, PPB:BLK, :],
                ).then_inc(s_yo[b % OR][1], 16)
            for s in range(OR):
                n_uses = len([bb for bb in range(N_BLK) if bb % OR == s])
                for h in range(2):
                    sync.wait_ge(s_yo[s][h], 16 * n_uses)

        def _evac_dve(v, b):
            # row-pairs 7..13 of block b
            ob = outb[b % OR]
            for p in range(7, PPB):
                gp = b * PPB + p
                if b >= OR and p == 7:
                    v.wait_ge(s_yo[b % OR][1], 16 * ((b - OR) // OR + 1))
                v.wait_ge(s_mm, gp + 1)
                v.tensor_scalar_add(
                    ob[:, 2 * p : 2 * p + 2, :],
                    ps[gp % NPS][:, :, :],
                    bias_sb[:, :],
                ).then_inc(s_evv, 1)

        @block.vector
        def _(v):
            v.wait_ge(s_bias, 16)
            # copies for block b run first (inputs arrive well ahead), then
            # evacs of block b-1 drain as PE produces them -- so a late
            # input can only delay PE block b's start, never PSUM recycling.
            for b in range(N_BLK):
                r = b % XR
                for img in range(IMGS_PER_CORE):
                    v.wait_ge(s_xb[img][r], 16 * len(DMA_PIECES) * (b // XR + 1))
                for img in range(IMGS_PER_CORE):
                    # dy=2 im2col replica: xb[64:96, s] = xb[32:64, s+1]
                    v.tensor_copy(
                        out=xb[img][r][64:96, 0:BLK, :],
                        in_=xb[img][r][32:64, 1 : BLK + 1, :],
                    ).then_inc(s_cp, 1)
                if b >= 1:
                    _evac_dve(v, b - 1)
            _evac_dve(v, N_BLK - 1)

        @block.tensor
        def _(t):
            t.wait_ge(s_wt, 16)
            for b in range(N_BLK):
                r = b % XR
                for img in range(IMGS_PER_CORE):
                    t.wait_ge(s_xb[img][r], 16 * len(DMA_PIECES) * (b // XR + 1))
                t.wait_ge(s_cp, IMGS_PER_CORE * (b + 1))
                for p in range(PPB):
                    gp = b * PPB + p
                    if gp >= NPS:
                        # psum bank reuse: evac of row-pair gp-NPS done
                        tb, tp = divmod(gp - NPS, PPB)
                        if tp < 7:
                            t.wait_ge(s_evs, EVS_PPB * tb + tp + 1)
                        else:
                            t.wait_ge(s_evv, EVV_PPB * tb + tp - 6)
                    bank = ps[gp % NPS]
                    b0 = 2 * p
                    last = None
                    for dx in range(3):
                        for img in range(IMGS_PER_CORE):
                            last = nc.tensor.matmul(
                                bank[img * OC : (img + 1) * OC, :, :],
                                wt_sb[:, dx, :],
                                xb[img][r][:, b0 : b0 + 2, dx : dx + W],
                                start=dx == 0,
                                stop=dx == 2,
                                skip_group_check=True,
                            )
                    last.then_inc(s_mm, 1)

        @block.scalar
        def _(sc):
            sc.wait_ge(s_bias, 16)
            for b in range(N_BLK):
                ob = outb[b % OR]
                for p in range(0, 7):
                    gp = b * PPB + p
                    if b >= OR and p == 0:
                        sc.wait_ge(s_yo[b % OR][0], 16 * ((b - OR) // OR + 1))
                    sc.wait_ge(s_mm, gp + 1)
                    sc.activation(
                        ob[:, 2 * p : 2 * p + 2, :],
                        ps[gp % NPS][:, :, :],
                        mybir.ActivationFunctionType.Identity,
                        bias=bias_sb[:, :],
                    ).then_inc(s_evs, 1)

    return nc


def _get_nc() -> bass.Bass:
    if "nc" not in _nc_cache:
        _nc_cache["nc"] = _build_nc()
    return _nc_cache["nc"]


def kernel(x: np.ndarray, weight: np.ndarray, bias: np.ndarray) -> np.ndarray:
    global LAST_EXEC_NS, LAST_RESULTS
    import ml_dtypes

    n = x.shape[0]
    assert n == N_CORES * IMGS_PER_CORE

    in_np = ml_dtypes.bfloat16
    xp = np.zeros((n, IC, HP, WP), dtype=in_np)
    xp[:, :, 1 : H + 1, 1 : W + 1] = x
    # WT[dy*32+ic, dx, oc] = weight[oc, ic, dy, dx]
    wt = np.ascontiguousarray(weight.transpose(2, 1, 3, 0).reshape(96, 3, OC)).astype(
        in_np
    )
    b2 = np.ascontiguousarray(np.tile(bias.reshape(OC, 1), (2, 1))).astype(np.float32)

    nc = _get_nc()
    in_maps = [
        {
            "x": np.ascontiguousarray(xp[i * IMGS_PER_CORE : (i + 1) * IMGS_PER_CORE]),
            "wt": wt,
            "bias": b2,
        }
        for i in range(N_CORES)
    ]
    if TRACE:
        _install_ntff_shim()
    res = run_bass_kernel_spmd(nc, in_maps, core_ids=list(range(N_CORES)), trace=TRACE)
    LAST_EXEC_NS = res.exec_time_ns
    LAST_RESULTS = res
    y = np.concatenate([r["y"] for r in res.results], axis=0)
    return y.astype(np.float32)
